# revision 39
# baseline (speedup 1.0000x reference)
"""DynamicFilter Trainium2 kernel.

Computation (per sample b):
    h  = tanh(query @ W1.T + b1)                      [B, 256]
    cw = (h @ W2.T + b2).reshape(B, C=32, K=31)       per-sample conv weights
    x[b,t,c] = sum_k cw[b,c,k] * pad(prev_attn)[b, t+k]
    out[b,t,o] = sum_c Wfc[o,c] x[b,t,c] + bfc[o]

Key algebraic fusion: fold the fc into the conv,
    Weff[b,o,k] = sum_c Wfc[o,c] cw[b,c,k]            [B, 128, 31]
    out[b,t,o]  = sum_k Weff[b,o,k] pad(prev_attn)[b, t+k] + bfc[o]
so the T-sized work is ONE fp32r matmul per (sample, 512-wide t-chunk):
    psum[128 o, 512 t] = WeffT_b[31 k, 128 o].T @ windows[31 k, 512 t]
with the windows operand streamed from SBUF tiles holding 31 shifted
replicas of each padded row (so every rhs is a plain rectangular slice).

The replica array is laid out host-side, 3 samples per 96-partition tile
at partition bases {0, 32, 64} (matmul operands may only start at those
bases), and loaded with one clean rectangular DMA per group -- the shape
this runtime's HWDGE actually distributes across all 16 SDMA engines.
fp32r operands are either produced by an on-chip rounding copy or
declared float32r at the DRAM tensor (both satisfy the walrus verifier's
rounding-chain check; fp32r rounding happens in the PE datapath).

Sharding: data-parallel over batch. 64 samples / 8 cores = 8 per core.
Weights replicated. Output written [b, o, t] (2 KB contiguous DMA runs),
host returns a transposed view [B, T, O].
"""

import sys

import numpy as np

if "/opt/trn_rl_repo" not in sys.path:
    sys.path.insert(0, "/opt/trn_rl_repo")

from contextlib import ExitStack

import concourse.bass as bass
import concourse.mybir as mybir
import concourse.tile as tile
from concourse import bacc
from concourse.ap import AP
from concourse.bass_utils import run_bass_kernel_spmd
from concourse.masks import make_identity

# Problem shapes (hardcoded per contract).
B, T = 64, 4096
D, H = 1024, 256
C, K, O = 32, 31, 128
PAD = (K - 1) // 2  # 15
NCORES = 8
BPC = B // NCORES  # 8 samples per core
TCH = 512  # t-chunk (matmul moving free dim)
NT = T // TCH  # 8 chunks per sample
GROUPS = [(0, 3), (3, 3), (6, 2)]  # (first sample, count) per replica tile

F32 = mybir.dt.float32
F32R = mybir.dt.float32r
BF16 = mybir.dt.bfloat16
AF = mybir.ActivationFunctionType

_CACHED = {}


def _build_nc(use_f32r=True):
    nc = bacc.Bacc(
        "TRN2", target_bir_lowering=False, debug=False, num_devices=NCORES
    )
    mmdt = F32R if use_f32r else F32

    # host-prepacked layouts: single contiguous DMAs into the exact SBUF
    # images (descriptor count on the HWDGE rings is a scarce resource)
    qT_h = nc.dram_tensor("qtp", [128, 8 * BPC], F32, kind="ExternalInput")
    rep_h = nc.dram_tensor("paRep", [len(GROUPS), 96, T], F32,
                           kind="ExternalInput")
    w1t_h = nc.dram_tensor("w1tp", [128, 8 * H], F32, kind="ExternalInput")
    b1_h = nc.dram_tensor("b1p", [128, 2], F32, kind="ExternalInput")
    w2t_h = nc.dram_tensor("w2tp", [128, 2 * C * K], F32, kind="ExternalInput")
    b2_h = nc.dram_tensor("b2", [C * K], F32, kind="ExternalInput")
    wfct_h = nc.dram_tensor("wfct", [C, O], F32, kind="ExternalInput")
    bfc_h = nc.dram_tensor("bfc", [O], F32, kind="ExternalInput")
    out_h = nc.dram_tensor("out", [BPC, O, T], F32, kind="ExternalOutput")

    with tile.TileContext(nc) as tc:
        _emit(tc, qT_h, rep_h, w1t_h, b1_h, w2t_h, b2_h, wfct_h, bfc_h, out_h,
              use_f32r)

    nc.compile()
    return nc


def _emit(tc, qT_h, rep_h, w1t_h, b1_h, w2t_h, b2_h, wfct_h, bfc_h, out_h,
          use_f32r):
    nc = tc.nc
    mmdt = F32R if use_f32r else F32
    with ExitStack() as ctx:
        singles = ctx.enter_context(tc.tile_pool(name="singles", bufs=1))
        cw_pool = ctx.enter_context(tc.tile_pool(name="cw", bufs=BPC))
        weff_pool = ctx.enter_context(tc.tile_pool(name="weff", bufs=3))
        pa_pool = ctx.enter_context(tc.tile_pool(name="pa", bufs=3))
        out_pool = ctx.enter_context(tc.tile_pool(name="outsb", bufs=6))
        psum_pre = ctx.enter_context(
            tc.tile_pool(name="psum_pre", bufs=2, space="PSUM")
        )
        psum_weff = ctx.enter_context(
            tc.tile_pool(name="psum_weff", bufs=2, space="PSUM")
        )
        psum_main = ctx.enter_context(
            tc.tile_pool(name="psum_main", bufs=4, space="PSUM")
        )

        # ---- staging: everything in ~2KB-descriptor chunks (the per-byte
        # sweet spot on this runtime's SDMA engines; 8-16KB descriptors
        # are ~1us latency-bound each).  Weights on the scalar ring,
        # replicas on the sync ring -- engines round-robin between the
        # two queue rows at packet granularity so both stream at once.
        rep_ap = rep_h.ap()
        pa_tiles = [
            pa_pool.tile([96, T], mmdt, tag="pa", name=f"pa_g{g}")
            for g in range(len(GROUPS))
        ]

        # qt_sb[p, (dc, b)] = qT[128*dc + p, b]  (small, needed first)
        qt_sb = singles.tile([128, 8 * BPC], mmdt)
        nc.scalar.dma_start(qt_sb[:], qT_h.ap().bitcast(mmdt))
        # w1t chunks as separate tiles: each mm1 d-chunk starts as soon as
        # its own 512-column chunk lands (whole-tile deps otherwise)
        w1t_tiles = []
        for ch in range(4):
            w1c = singles.tile([128, 512], mmdt, name=f"w1c{ch}")
            eng = nc.sync if ch % 2 == 0 else nc.scalar
            eng.dma_start(
                w1c[:],
                w1t_h.ap()[:, 512 * ch : 512 * ch + 512].bitcast(mmdt),
            )
            w1t_tiles.append(w1c)
        # w2t chunks likewise; chunk (2*hc + nh) is exactly one mm2 operand
        w2t_tiles = []
        for ch in range(4):
            w2c = singles.tile([128, 496], mmdt, name=f"w2c{ch}")
            eng = nc.sync if ch % 2 == 0 else nc.scalar
            eng.dma_start(
                w2c[:],
                w2t_h.ap()[:, 496 * ch : 496 * ch + 496].bitcast(mmdt),
            )
            w2t_tiles.append(w2c)
        wfct_sb = singles.tile([C, O], F32)
        nc.scalar.dma_start(wfct_sb[:], wfct_h.ap())
        b1_sb = singles.tile([128, 2], F32)
        nc.scalar.dma_start(b1_sb[:], b1_h.ap())
        b2_sb = singles.tile([1, C * K], mmdt)
        nc.scalar.dma_start(b2_sb[:], b2_h.ap().unsqueeze(0).bitcast(mmdt))
        bfc_sb = singles.tile([O, 1], F32)
        nc.scalar.dma_start(bfc_sb[:], bfc_h.ap().unsqueeze(1))
        # replica groups in 2KB column chunks on the sync ring
        for gi, (b0, cnt) in enumerate(GROUPS):
            for ch in range(NT):
                nc.sync.dma_start(
                    pa_tiles[gi][0 : 32 * cnt, TCH * ch : TCH * ch + TCH],
                    rep_ap[gi, 0 : 32 * cnt,
                           TCH * ch : TCH * ch + TCH].bitcast(mmdt),
                )
        ones_f32 = singles.tile([1, BPC], F32)
        nc.gpsimd.memset(ones_f32[:], 1.0)
        ones_sb = singles.tile([1, BPC], mmdt)
        nc.vector.tensor_copy(ones_sb[:], ones_f32[:])
        ident_sb = singles.tile([BPC, BPC], F32)
        make_identity(nc, ident_sb[:])

        # ---- hypernet mm1 (wide-N orientation): h[b, j] --------------
        # h[b, j] = sum_d qT[d, b] W1T[d, j]; N=256 so fp32r streams at
        # full rate; tanh applied on the copy out of PSUM
        ph = psum_pre.tile([BPC, H], F32, tag="pre")
        for dc in range(8):
            nc.tensor.matmul(
                ph[:],
                lhsT=qt_sb[:, BPC * dc : BPC * dc + BPC],
                rhs=w1t_tiles[dc // 2][:, H * (dc % 2) : H * (dc % 2) + H],
                start=(dc == 0),
                stop=(dc == 7),
            )
        h_sb = singles.tile([BPC, H], F32)
        # bias varies along the free dim here -> add via DVE broadcast of
        # b1 is not possible; instead bias rows are folded with tanh by
        # scalar engine per 128-column half using activation's bias on the
        # TRANSPOSED layout below.  b1 is zeros in practice but handled
        # exactly: tanh(x + b1) computed after transpose, so copy raw here.
        nc.vector.tensor_copy(h_sb[:], ph[:])

        # transpose h -> hT chunks [128 j, BPC] and apply tanh(+b1) there
        ht_sb = singles.tile([128, 2 * BPC], F32)
        for jc in range(2):
            pt = psum_pre.tile([128, BPC], F32, tag="pre")
            nc.tensor.transpose(
                pt[:], h_sb[:, 128 * jc : 128 * jc + 128], ident_sb[:]
            )
            nc.scalar.activation(
                ht_sb[:, BPC * jc : BPC * jc + BPC], pt[:], AF.Tanh,
                bias=b1_sb[:, jc : jc + 1],
            )
        # rounded copy for the fp32r mm2 stationary
        htr_sb = singles.tile([128, 2 * BPC], mmdt)
        nc.vector.tensor_copy(htr_sb[:], ht_sb[:])

        # ---- hypernet mm2: cwB[b, (c k)] = sum_h W2T[h, ck] hT[h, b] + b2 --
        cwB_sb = singles.tile([BPC, C * K], F32)
        HALF = C * K // 2  # 496
        for nh in range(2):
            pc = psum_pre.tile([BPC, HALF], F32, tag="pre")
            for hc in range(2):
                nc.tensor.matmul(
                    pc[:],
                    lhsT=htr_sb[:, BPC * hc : BPC * hc + BPC],
                    rhs=w2t_tiles[2 * hc + nh][:],
                    start=(hc == 0),
                    stop=False,
                )
            # bias row: K=1 accumulating matmul with a ones stationary
            nc.tensor.matmul(
                pc[:],
                lhsT=ones_sb[:],
                rhs=b2_sb[:, HALF * nh : HALF * nh + HALF],
                start=False,
                stop=True,
            )
            nc.vector.tensor_copy(cwB_sb[:, HALF * nh : HALF * nh + HALF], pc[:])

        # ---- per-sample cw gather: cw_b[c, k] <- cwB[b, 31c + k] ------
        cw_tiles = []
        for b in range(BPC):
            cwt = cw_pool.tile([C, K], F32, tag="cwt")
            eng = nc.scalar if b % 2 == 0 else nc.sync
            eng.dma_start(
                cwt[:], cwB_sb[b : b + 1, :].rearrange("p (c k) -> p c k", c=C)
            )
            cw_tiles.append(cwt)

        # ---- Weff per group: WeffT_b[k, o] = sum_c cw_b[c, k] WfcT[c, o] ---
        # sample i of a group lives at partition base 32*i
        weff_tiles = []
        for b0, cnt in GROUPS:
            pw = psum_weff.tile([96, O], F32, tag="pweff")
            for i in range(cnt):
                nc.tensor.matmul(
                    pw[32 * i : 32 * i + K, :],
                    lhsT=cw_tiles[b0 + i][:],
                    rhs=wfct_sb[:],
                    start=True,
                    stop=True,
                )
            wg = weff_pool.tile([96, O], mmdt, tag="weff")
            for i in range(cnt):
                nc.vector.tensor_copy(
                    wg[32 * i : 32 * i + K, :], pw[32 * i : 32 * i + K, :]
                )
            weff_tiles.append(wg)

        # ---- main loop ------------------------------------------------
        idx = 0
        out_ap = out_h.ap()
        for gi, (b0, cnt) in enumerate(GROUPS):
            pa_g = pa_tiles[gi]
            wg = weff_tiles[gi]
            for i in range(cnt):
                lhsT = wg[32 * i : 32 * i + K, :]
                b = b0 + i
                for tcn in range(NT):
                    pm = psum_main.tile([O, TCH], F32, tag="pmm")
                    nc.tensor.matmul(
                        pm[:],
                        lhsT=lhsT,
                        rhs=pa_g[32 * i : 32 * i + K, TCH * tcn : TCH * tcn + TCH],
                        start=True,
                        stop=True,
                    )
                    osb = out_pool.tile([O, TCH], F32, tag="osb")
                    # psum -> sbuf with +bfc, 5:3 DVE:ACT (ACT also pays
                    # for half the out dispatches); out-DMAs alternate
                    # between the two HWDGE rings
                    if idx % 8 < 5:
                        nc.vector.tensor_scalar_add(osb[:], pm[:],
                                                    bfc_sb[:, 0:1])
                    else:
                        nc.scalar.activation(osb[:], pm[:], AF.Identity,
                                             bias=bfc_sb[:, 0:1])
                    eng = nc.sync if idx % 2 == 0 else nc.scalar
                    eng.dma_start(
                        out_ap[b, :, TCH * tcn : TCH * tcn + TCH], osb[:]
                    )
                    idx += 1


def get_nc(use_f32r=True):
    key = ("nc", use_f32r)
    if key not in _CACHED:
        _CACHED[key] = _build_nc(use_f32r)
    return _CACHED[key]


def make_in_maps(query, prev_attn, W1, b1, W2, b2, Wfc, bfc):
    """Shard + lay out host inputs for the 8 cores."""
    f = np.float32
    w1t = np.asarray(W1, f).T  # [D, H]
    w2t = np.asarray(W2, f).T  # [H, C*K]
    wfct = np.ascontiguousarray(np.asarray(Wfc, f).T)  # [C, O]
    b1 = np.asarray(b1, f)
    b2 = np.ascontiguousarray(np.asarray(b2, f))
    bfc = np.ascontiguousarray(np.asarray(bfc, f))
    query = np.asarray(query, f)
    prev_attn = np.asarray(prev_attn, f)

    # prepack into the SBUF partition-major images the kernel DMAs verbatim
    # w1tp[p, (dc, j)] = W1T[128*dc + p, j]
    w1tp = np.ascontiguousarray(
        w1t.reshape(8, 128, H).transpose(1, 0, 2).reshape(128, 8 * H)
    )
    w2tp = np.ascontiguousarray(
        w2t.reshape(2, 128, C * K).transpose(1, 0, 2).reshape(128, 2 * C * K)
    )
    b1p = np.ascontiguousarray(b1.reshape(2, 128).T)  # [128, 2]

    in_maps = []
    for i in range(NCORES):
        sl = slice(i * BPC, (i + 1) * BPC)
        qT = query[sl].T  # [D, BPC]
        qtp = np.ascontiguousarray(
            qT.reshape(8, 128, BPC).transpose(1, 0, 2).reshape(128, 8 * BPC)
        )
        # shifted replicas: paRep[g, 32*i + k, t] = pad(prev_attn)[b0+i, k+t]
        padded = np.zeros((BPC, T + 2 * PAD), f)
        padded[:, PAD : PAD + T] = prev_attn[sl]
        win = np.lib.stride_tricks.sliding_window_view(padded, T, axis=1)
        # win[b, k, t] = padded[b, k + t], k in [0, 31)
        rep = np.zeros((len(GROUPS), 96, T), f)
        for g, (b0, cnt) in enumerate(GROUPS):
            for j in range(cnt):
                rep[g, 32 * j : 32 * j + K] = win[b0 + j]
        in_maps.append(
            {
                "qtp": qtp,
                "paRep": rep,
                "w1tp": w1tp,
                "b1p": b1p,
                "w2tp": w2tp,
                "b2": b2,
                "wfct": wfct,
                "bfc": bfc,
            }
        )
    return in_maps


def assemble_output(results):
    """[8 cores] x [BPC, O, T] -> [B, T, O] view."""
    full = np.concatenate([r["out"] for r in results], axis=0)  # [B, O, T]
    return full.transpose(0, 2, 1)


def kernel(query, prev_attn, W1, b1, W2, b2, Wfc, bfc):
    nc = get_nc(use_f32r=True)
    in_maps = make_in_maps(query, prev_attn, W1, b1, W2, b2, Wfc, bfc)
    res = run_bass_kernel_spmd(nc, in_maps, list(range(NCORES)))
    return assemble_output(res.results)
